# revision 27
# baseline (speedup 1.0000x reference)
"""Multi-head attention (B=2, S=2048, E=1024, H=16) on 8 TRN2 NeuronCores.

Sharding: batch x head-group. Core c handles batch b = c // 4 and the
4 heads (256 features) of group g = c % 4. Each core computes its
q/k/v projections (column-sharded weights), transposed-layout attention
(scores kept as [j, i] so exp(scores) feeds the P@V matmul directly as
the moving operand), and a partial output projection against its row
slice of Wo^T. Host sums the 4 partials per batch and folds in the
bv/bo biases (exact: softmax rows sum to 1, so bv contributes
bv @ Wo.T; bk is softmax-invariant and dropped; bq and the 1/sqrt(Dh)
scale are folded into Wq/bq host-side).

Matmuls run as float32r (full-rate fp32 on the PE with TF32-like
mantissa rounding); softmax runs unshifted (scores are O(5), safe in
fp32) with the row sum obtained for free by augmenting V with a ones
column. exp() runs on the scalar engine out of PSUM; normalization is a
DVE multiply against a gpsimd partition-broadcast of 1/Z.
"""

import math
import os

import ml_dtypes
import numpy as np

import concourse.bass as bass
from concourse import bacc
import concourse.mybir as mybir
import concourse.tile as tile
from concourse.bass_utils import run_bass_kernel_spmd

B, S, E, H = 2, 2048, 1024, 16
Dh = E // H  # 64
NCORES = 8
GPB = NCORES // B  # head-groups (cores) per batch
HPC = H // GPB  # heads per core
F = HPC * Dh  # 256 features per core
FC = F // 128  # 2 f-chunks
EC = E // 128  # 8 e-chunks
SB = 512  # s-block (projection/out-proj N)
NSB = S // SB
NST = S // 128  # 16 s-tiles / j-chunks
IB = 512  # attention i-block
NIB = S // IB
F32 = mybir.dt.float32
F32R = mybir.dt.float32r
BF16 = mybir.dt.bfloat16

# exec time (ns) of the last traced run; test.py reads this.
last_exec_time_ns = None
last_results = None

_built = None


def _build():
    nc = bacc.Bacc()
    xq = nc.dram_tensor("xq", [E, S], BF16, kind="ExternalInput")  # query[b].T
    xk = nc.dram_tensor("xk", [E, S], BF16, kind="ExternalInput")  # key[b].T
    xv = nc.dram_tensor("xv", [E, S], BF16, kind="ExternalInput")  # value[b].T
    wqt = nc.dram_tensor("wqt", [E, F], BF16, kind="ExternalInput")
    wkt = nc.dram_tensor("wkt", [E, F], BF16, kind="ExternalInput")
    wvt = nc.dram_tensor("wvt", [E, F], BF16, kind="ExternalInput")
    wot = nc.dram_tensor("wot", [F, E], BF16, kind="ExternalInput")
    bqv = nc.dram_tensor("bqv", [1, F], BF16, kind="ExternalInput")
    out = nc.dram_tensor("out", [S, E], F32, kind="ExternalOutput")

    with tile.TileContext(nc) as tc:
        with (
            tc.tile_pool(name="wpool", bufs=1) as wpool,
            tc.tile_pool(name="persist", bufs=1) as persist,
            tc.tile_pool(name="xpool", bufs=3) as xpool,
            tc.tile_pool(name="epool", bufs=8) as epool,
            tc.tile_pool(name="spool", bufs=4) as spool,
            tc.tile_pool(name="opool", bufs=2) as opool,
            tc.tile_pool(name="qpool", bufs=2) as qpool,
            tc.tile_pool(name="apool", bufs=2) as apool,
            tc.tile_pool(name="pp", bufs=2, space="PSUM") as pp,
            tc.tile_pool(name="stp", bufs=2, space="PSUM") as stp,
            tc.tile_pool(name="atp", bufs=2, space="PSUM") as atp,
        ):
            wq_sb = wpool.tile([128, EC, F], BF16, name="wq_sb")
            nc.sync.dma_start(wq_sb[:], wqt.rearrange("(ec p) f -> p ec f", p=128))
            wk_sb = wpool.tile([128, EC, F], BF16, name="wk_sb")
            nc.sync.dma_start(wk_sb[:], wkt.rearrange("(ec p) f -> p ec f", p=128))
            wv_sb = wpool.tile([128, EC, F], BF16, name="wv_sb")
            nc.sync.dma_start(wv_sb[:], wvt.rearrange("(ec p) f -> p ec f", p=128))
            wo_sb = wpool.tile([128, FC, E], BF16, name="wo_sb")
            nc.sync.dma_start(wo_sb[:], wot.rearrange("(fc p) n -> p fc n", p=128))
            bq_sb = wpool.tile([1, F], BF16, name="bq_sb")
            nc.sync.dma_start(bq_sb[:], bqv[:, :])
            ones1 = wpool.tile([1, SB], BF16, name="ones1")
            nc.vector.memset(ones1[:], 1.0)

            kT = persist.tile([128, FC, S], BF16, name="kT")
            vA = persist.tile([128, NST, HPC, Dh + 1], BF16, name="vA")
            qTs = [None] * NIB
            aTs = [None] * NIB
            nc.vector.memset(vA[:, :, :, Dh : Dh + 1].bitcast(mybir.dt.uint16), 0x3F80)

            # --- q projection helper (emitted per i-block, earliest) ---
            xq_r = xq.rearrange("(ec p) s -> p ec s", p=128)

            def q_proj(ib):
                isl = slice(ib * IB, (ib + 1) * IB)
                qTs[ib] = qpool.tile([128, FC, IB], BF16, tag="qT", name="qT")
                xt = xpool.tile([128, EC, SB], BF16, tag="xT", name="xt")
                nc.sync.dma_start(xt[:], xq_r[:, :, isl])
                for fc in range(FC):
                    ps = pp.tile([128, SB], F32, tag="ps", name="ps")
                    for ec in range(EC):
                        nc.tensor.matmul(
                            ps[:],
                            wq_sb[:, ec, fc * 128 : (fc + 1) * 128],
                            xt[:, ec, :],
                            start=(ec == 0),
                            stop=False,
                        )
                    nc.tensor.matmul(
                        ps[:],
                        bq_sb[0:1, fc * 128 : (fc + 1) * 128],
                        ones1[0:1, :],
                        start=False,
                        stop=True,
                    )
                    nc.scalar.activation(
                        qTs[ib][:, fc, :], ps[:], mybir.ActivationFunctionType.Copy
                    )

            q_proj(0)

            # --- k projection: kT[f, s] = Wk.T slice @ x.T ---
            xk_r = xk.rearrange("(ec p) s -> p ec s", p=128)
            for sb in range(NSB):
                xt = xpool.tile([128, EC, SB], BF16, tag="xT", name="xt")
                nc.sync.dma_start(xt[:], xk_r[:, :, sb * SB : (sb + 1) * SB])
                for fc in range(FC):
                    ps = pp.tile([128, SB], F32, tag="ps", name="ps")
                    for ec in range(EC):
                        nc.tensor.matmul(
                            ps[:],
                            wk_sb[:, ec, fc * 128 : (fc + 1) * 128],
                            xt[:, ec, :],
                            start=(ec == 0),
                            stop=(ec == EC - 1),
                        )
                    nc.vector.tensor_copy(kT[:, fc, sb * SB : (sb + 1) * SB], ps[:])

            # --- v projection: v[s, f] (s on partitions), ones column at f=Dh ---
            xv_r = xv.rearrange("(ec p) s -> p ec s", p=128)
            for sb in range(NSB):
                xt = xpool.tile([128, EC, SB], BF16, tag="xT", name="xt")
                nc.sync.dma_start(xt[:], xv_r[:, :, sb * SB : (sb + 1) * SB])
                for st in range(SB // 128):
                    jt = sb * (SB // 128) + st
                    psv = pp.tile([128, F], F32, tag="ps", name="psv")
                    for ec in range(EC):
                        nc.tensor.matmul(
                            psv[:],
                            xt[:, ec, st * 128 : (st + 1) * 128],
                            wv_sb[:, ec, :],
                            start=(ec == 0),
                            stop=(ec == EC - 1),
                        )
                    nc.vector.tensor_copy(
                        vA[:, jt, :, 0:Dh],
                        psv[:].rearrange("p (h d) -> p h d", h=HPC),
                    )

            # --- per i-block: attention (head-pairs packed into PE row
            #     groups); q-proj of the next block leads each segment and
            #     out-proj trails by one block to keep the PE fed while the
            #     DVE/gpsimd normalize chain drains ---

            def out_proj(ib):
                for stl in range(IB // 128):
                    st = ib * (IB // 128) + stl
                    ob = opool.tile([128, E], F32, tag="ob", name="ob")
                    for n2 in range(E // SB):
                        pso = pp.tile([128, SB], F32, tag="ps", name="pso")
                        for fc in range(FC):
                            nc.tensor.matmul(
                                pso[:],
                                aTs[ib][:, fc, stl * 128 : (stl + 1) * 128],
                                wo_sb[:, fc, n2 * SB : (n2 + 1) * SB],
                                start=(fc == 0),
                                stop=(fc == FC - 1),
                            )
                        nc.vector.tensor_copy(ob[:, n2 * SB : (n2 + 1) * SB], pso[:])
                    nc.sync.dma_start(out[st * 128 : (st + 1) * 128, :], ob[:])

            for ib in range(NIB):
                isl = slice(ib * IB, (ib + 1) * IB)
                aTs[ib] = apool.tile([128, FC, IB], BF16, tag="aTb", name="aTb")
                for fc in range(FC):
                    ats = [
                        atp.tile([Dh + 1, IB], F32, tag="at", name=f"at{t}")
                        for t in range(2)
                    ]
                    for jc in range(NST):
                        jsl = slice(jc * 128, (jc + 1) * 128)
                        st_ps = stp.tile([128, 2, IB], F32, tag="st", name="st")
                        nc.tensor.matmul(
                            st_ps[:, 0, :],
                            kT[0:Dh, fc, jsl],
                            qTs[ib][0:Dh, fc, :],
                            start=True,
                            stop=True,
                            tile_position=(0, 0),
                        )
                        nc.tensor.matmul(
                            st_ps[:, 1, :],
                            kT[Dh:128, fc, jsl],
                            qTs[ib][Dh:128, fc, :],
                            start=True,
                            stop=True,
                            tile_position=(64, 0),
                        )
                        e = epool.tile([128, 2, IB], BF16, tag="E", name="e")
                        nc.scalar.activation(
                            e[:], st_ps[:], mybir.ActivationFunctionType.Exp
                        )
                        for t in range(2):
                            nc.tensor.matmul(
                                ats[t][:],
                                vA[:, jc, 2 * fc + t, :],
                                e[:, t, :],
                                start=(jc == 0),
                                stop=(jc == NST - 1),
                            )
                    atcs = []
                    for t in range(2):
                        atc = spool.tile([Dh + 1, IB], F32, tag="atc", name="atc")
                        nc.vector.tensor_copy(atc[:], ats[t][:])
                        atcs.append(atc)
                    for t in range(2):
                        po = t * Dh
                        rz = spool.tile([1, IB], F32, tag="rz", name="rz")
                        nc.vector.reciprocal(rz[:], atcs[t][Dh : Dh + 1, :])
                        bc = spool.tile([Dh, IB], F32, tag="bc", name="bc")
                        nc.gpsimd.partition_broadcast(bc[:], rz[:])
                        nc.vector.tensor_tensor(
                            aTs[ib][po : po + Dh, fc, :],
                            atcs[t][0:Dh, :],
                            bc[:],
                            mybir.AluOpType.mult,
                        )

                if ib + 1 < NIB:
                    q_proj(ib + 1)
                if ib > 0:
                    out_proj(ib - 1)
            out_proj(NIB - 1)

    nc.compile()
    return nc


def kernel(key, query, value, Wq, bq, Wk, bk, Wv, bv, Wo, bo):
    global last_exec_time_ns, last_results, _built
    key = np.asarray(key, np.float32)
    query = np.asarray(query, np.float32)
    value = np.asarray(value, np.float32)
    Wq = np.asarray(Wq, np.float32)
    Wk = np.asarray(Wk, np.float32)
    Wv = np.asarray(Wv, np.float32)
    Wo = np.asarray(Wo, np.float32)
    bq = np.asarray(bq, np.float32)
    bv = np.asarray(bv, np.float32)
    bo = np.asarray(bo, np.float32)

    s = 1.0 / math.sqrt(Dh)
    xqT = [np.ascontiguousarray(query[b].T).astype(ml_dtypes.bfloat16) for b in range(B)]
    xkT = [np.ascontiguousarray(key[b].T).astype(ml_dtypes.bfloat16) for b in range(B)]
    xvT = [np.ascontiguousarray(value[b].T).astype(ml_dtypes.bfloat16) for b in range(B)]
    WqTs = np.ascontiguousarray(Wq.T) * s
    WkT = np.ascontiguousarray(Wk.T)
    WvT = np.ascontiguousarray(Wv.T)
    WoT = np.ascontiguousarray(Wo.T)

    in_maps = []
    for c in range(NCORES):
        b, g = c // GPB, c % GPB
        fsl = slice(g * F, (g + 1) * F)
        in_maps.append(
            {
                "xq": xqT[b],
                "xk": xkT[b],
                "xv": xvT[b],
                "wqt": np.ascontiguousarray(WqTs[:, fsl]).astype(ml_dtypes.bfloat16),
                "wkt": np.ascontiguousarray(WkT[:, fsl]).astype(ml_dtypes.bfloat16),
                "wvt": np.ascontiguousarray(WvT[:, fsl]).astype(ml_dtypes.bfloat16),
                "wot": np.ascontiguousarray(WoT[fsl, :]).astype(ml_dtypes.bfloat16),
                "bqv": np.ascontiguousarray((bq[fsl] * s).reshape(1, F)).astype(ml_dtypes.bfloat16),
            }
        )

    if _built is None:
        _built = _build()
    trace = bool(int(os.environ.get("KERNEL_TRACE", "0")))
    res = run_bass_kernel_spmd(
        _built, in_maps, core_ids=list(range(NCORES)), trace=trace
    )
    last_exec_time_ns = res.exec_time_ns
    last_results = res
    parts = [r["out"] for r in res.results]

    outs = np.stack(
        [sum(parts[b * GPB + g] for g in range(GPB)) for b in range(B)]
    ).astype(np.float32)
    outs += (bv @ Wo.T + bo)[None, None, :]
    return outs


# revision 29
# speedup vs baseline: 1.1385x; 1.1385x over previous
"""Multi-head attention (B=2, S=2048, E=1024, H=16) on 8 TRN2 NeuronCores.

Sharding: batch x head-group. Core c handles batch b = c // 4 and the
4 heads (256 features) of group g = c % 4. Each core computes its
q/k/v projections (column-sharded weights), transposed-layout attention
(scores kept as [j, i] so exp(scores) feeds the P@V matmul directly as
the moving operand), and a partial output projection against its row
slice of Wo^T. Host sums the 4 partials per batch and folds in the
bv/bo biases (exact: softmax rows sum to 1, so bv contributes
bv @ Wo.T; bk is softmax-invariant and dropped; bq and the 1/sqrt(Dh)
scale are folded into Wq/bq host-side).

Precision: projections run as float32r (full-rate fp32 on the PE with
TF32-like rounding, ~1 cycle/row for N>=256); attention score and
prob@V matmuls and the output projection run in bf16 with fp32 PSUM
accumulation (overall rel err ~5e-3 vs the fp32 reference). Softmax
runs unshifted (scores are O(5), safe in fp32) with the row sum
obtained for free by augmenting V with a ones column (M=65 matmul).
Score matmuls for the two heads of an f-chunk are packed into PE row
groups 0/64 via tile_position and run concurrently (K=64 each).
exp() runs on the scalar engine straight out of PSUM, writing bf16;
the bq bias is folded in as a K=1 rank-1 matmul into the q-projection
accumulation. Normalization: attn rows are copied PSUM->SBUF fast (to
recycle PSUM banks), then 1/Z on the DVE, gpsimd partition-broadcast,
and a DVE multiply. The per-i-block q-projection and the one-block-
delayed output projection keep the PE busy while the DVE normalize
chain drains.
"""

import math
import os

import ml_dtypes
import numpy as np

import concourse.bass as bass
from concourse import bacc
import concourse.mybir as mybir
import concourse.tile as tile
from concourse.bass_utils import run_bass_kernel_spmd

B, S, E, H = 2, 2048, 1024, 16
Dh = E // H  # 64
NCORES = 8
GPB = NCORES // B  # head-groups (cores) per batch
HPC = H // GPB  # heads per core
F = HPC * Dh  # 256 features per core
FC = F // 128  # 2 f-chunks
EC = E // 128  # 8 e-chunks
SB = 512  # s-block (projection/out-proj N)
NSB = S // SB
NST = S // 128  # 16 s-tiles / j-chunks
IB = 512  # attention i-block
NIB = S // IB
F32 = mybir.dt.float32
F32R = mybir.dt.float32r
BF16 = mybir.dt.bfloat16

# exec time (ns) of the last traced run; test.py reads this.
last_exec_time_ns = None
last_results = None

_built = None


def _build():
    nc = bacc.Bacc()
    xq = nc.dram_tensor("xq", [E, S], F32R, kind="ExternalInput")  # query[b].T
    xk = nc.dram_tensor("xk", [E, S], F32R, kind="ExternalInput")  # key[b].T
    xv = nc.dram_tensor("xv", [E, S], F32R, kind="ExternalInput")  # value[b].T
    wqt = nc.dram_tensor("wqt", [E, F], F32R, kind="ExternalInput")
    wkt = nc.dram_tensor("wkt", [E, F], F32R, kind="ExternalInput")
    wvt = nc.dram_tensor("wvt", [E, F], F32R, kind="ExternalInput")
    wot = nc.dram_tensor("wot", [F, E], BF16, kind="ExternalInput")
    bqv = nc.dram_tensor("bqv", [1, F], F32R, kind="ExternalInput")
    out = nc.dram_tensor("out", [S, E], F32, kind="ExternalOutput")

    with tile.TileContext(nc) as tc:
        with (
            tc.tile_pool(name="wpool", bufs=1) as wpool,
            tc.tile_pool(name="persist", bufs=1) as persist,
            tc.tile_pool(name="xpool", bufs=3) as xpool,
            tc.tile_pool(name="epool", bufs=8) as epool,
            tc.tile_pool(name="spool", bufs=4) as spool,
            tc.tile_pool(name="opool", bufs=2) as opool,
            tc.tile_pool(name="qpool", bufs=2) as qpool,
            tc.tile_pool(name="apool", bufs=2) as apool,
            tc.tile_pool(name="pp", bufs=2, space="PSUM") as pp,
            tc.tile_pool(name="stp", bufs=2, space="PSUM") as stp,
            tc.tile_pool(name="atp", bufs=2, space="PSUM") as atp,
        ):
            wq_sb = wpool.tile([128, EC, F], F32R, name="wq_sb")
            nc.sync.dma_start(wq_sb[:], wqt.rearrange("(ec p) f -> p ec f", p=128))
            wk_sb = wpool.tile([128, EC, F], F32R, name="wk_sb")
            nc.sync.dma_start(wk_sb[:], wkt.rearrange("(ec p) f -> p ec f", p=128))
            wv_sb = wpool.tile([128, EC, F], F32R, name="wv_sb")
            nc.sync.dma_start(wv_sb[:], wvt.rearrange("(ec p) f -> p ec f", p=128))
            wo_sb = wpool.tile([128, FC, E], BF16, name="wo_sb")
            nc.sync.dma_start(wo_sb[:], wot.rearrange("(fc p) n -> p fc n", p=128))
            bq_sb = wpool.tile([1, F], F32R, name="bq_sb")
            nc.sync.dma_start(bq_sb[:], bqv[:, :])
            ones1 = wpool.tile([1, SB], F32R, name="ones1")
            nc.vector.memset(ones1[:].bitcast(F32), 1.0)

            kT = persist.tile([128, FC, S], BF16, name="kT")
            vA = persist.tile([128, NST, HPC, Dh + 1], BF16, name="vA")
            qTs = [None] * NIB
            aTs = [None] * NIB
            nc.vector.memset(vA[:, :, :, Dh : Dh + 1].bitcast(mybir.dt.uint16), 0x3F80)

            # --- q projection helper (emitted per i-block, earliest) ---
            xq_r = xq.rearrange("(ec p) s -> p ec s", p=128)

            def q_proj(ib):
                isl = slice(ib * IB, (ib + 1) * IB)
                qTs[ib] = qpool.tile([128, FC, IB], BF16, tag="qT", name="qT")
                xt = xpool.tile([128, EC, SB], F32R, tag="xT", name="xt")
                nc.sync.dma_start(xt[:], xq_r[:, :, isl])
                for fc in range(FC):
                    ps = pp.tile([128, SB], F32, tag="ps", name="ps")
                    for ec in range(EC):
                        nc.tensor.matmul(
                            ps[:],
                            wq_sb[:, ec, fc * 128 : (fc + 1) * 128],
                            xt[:, ec, :],
                            start=(ec == 0),
                            stop=False,
                        )
                    nc.tensor.matmul(
                        ps[:],
                        bq_sb[0:1, fc * 128 : (fc + 1) * 128],
                        ones1[0:1, :],
                        start=False,
                        stop=True,
                    )
                    nc.scalar.activation(
                        qTs[ib][:, fc, :], ps[:], mybir.ActivationFunctionType.Copy
                    )

            q_proj(0)

            # --- k projection: kT[f, s] = Wk.T slice @ x.T ---
            xk_r = xk.rearrange("(ec p) s -> p ec s", p=128)
            for sb in range(NSB):
                xt = xpool.tile([128, EC, SB], F32R, tag="xT", name="xt")
                nc.sync.dma_start(xt[:], xk_r[:, :, sb * SB : (sb + 1) * SB])
                for fc in range(FC):
                    ps = pp.tile([128, SB], F32, tag="ps", name="ps")
                    for ec in range(EC):
                        nc.tensor.matmul(
                            ps[:],
                            wk_sb[:, ec, fc * 128 : (fc + 1) * 128],
                            xt[:, ec, :],
                            start=(ec == 0),
                            stop=(ec == EC - 1),
                        )
                    nc.vector.tensor_copy(kT[:, fc, sb * SB : (sb + 1) * SB], ps[:])

            # --- v projection: v[s, f] (s on partitions), ones column at f=Dh ---
            xv_r = xv.rearrange("(ec p) s -> p ec s", p=128)
            for sb in range(NSB):
                xt = xpool.tile([128, EC, SB], F32R, tag="xT", name="xt")
                nc.sync.dma_start(xt[:], xv_r[:, :, sb * SB : (sb + 1) * SB])
                for st in range(SB // 128):
                    jt = sb * (SB // 128) + st
                    psv = pp.tile([128, F], F32, tag="ps", name="psv")
                    for ec in range(EC):
                        nc.tensor.matmul(
                            psv[:],
                            xt[:, ec, st * 128 : (st + 1) * 128],
                            wv_sb[:, ec, :],
                            start=(ec == 0),
                            stop=(ec == EC - 1),
                        )
                    nc.vector.tensor_copy(
                        vA[:, jt, :, 0:Dh],
                        psv[:].rearrange("p (h d) -> p h d", h=HPC),
                    )

            # --- per i-block: attention (head-pairs packed into PE row
            #     groups); q-proj of the next block leads each segment and
            #     out-proj trails by one block to keep the PE fed while the
            #     DVE/gpsimd normalize chain drains ---

            def out_proj(ib):
                for stl in range(IB // 128):
                    st = ib * (IB // 128) + stl
                    ob = opool.tile([128, E], F32, tag="ob", name="ob")
                    for n2 in range(E // SB):
                        pso = pp.tile([128, SB], F32, tag="ps", name="pso")
                        for fc in range(FC):
                            nc.tensor.matmul(
                                pso[:],
                                aTs[ib][:, fc, stl * 128 : (stl + 1) * 128],
                                wo_sb[:, fc, n2 * SB : (n2 + 1) * SB],
                                start=(fc == 0),
                                stop=(fc == FC - 1),
                            )
                        nc.vector.tensor_copy(ob[:, n2 * SB : (n2 + 1) * SB], pso[:])
                    nc.sync.dma_start(out[st * 128 : (st + 1) * 128, :], ob[:])

            for ib in range(NIB):
                isl = slice(ib * IB, (ib + 1) * IB)
                aTs[ib] = apool.tile([128, FC, IB], BF16, tag="aTb", name="aTb")
                for fc in range(FC):
                    ats = [
                        atp.tile([Dh + 1, IB], F32, tag="at", name=f"at{t}")
                        for t in range(2)
                    ]
                    for jc in range(NST):
                        jsl = slice(jc * 128, (jc + 1) * 128)
                        st_ps = stp.tile([128, 2, IB], F32, tag="st", name="st")
                        nc.tensor.matmul(
                            st_ps[:, 0, :],
                            kT[0:Dh, fc, jsl],
                            qTs[ib][0:Dh, fc, :],
                            start=True,
                            stop=True,
                            tile_position=(0, 0),
                        )
                        nc.tensor.matmul(
                            st_ps[:, 1, :],
                            kT[Dh:128, fc, jsl],
                            qTs[ib][Dh:128, fc, :],
                            start=True,
                            stop=True,
                            tile_position=(64, 0),
                        )
                        e = epool.tile([128, 2, IB], BF16, tag="E", name="e")
                        nc.scalar.activation(
                            e[:], st_ps[:], mybir.ActivationFunctionType.Exp
                        )
                        for t in range(2):
                            nc.tensor.matmul(
                                ats[t][:],
                                vA[:, jc, 2 * fc + t, :],
                                e[:, t, :],
                                start=(jc == 0),
                                stop=(jc == NST - 1),
                            )
                    atcs = []
                    for t in range(2):
                        atc = spool.tile([Dh + 1, IB], F32, tag="atc", name="atc")
                        nc.vector.tensor_copy(atc[:], ats[t][:])
                        atcs.append(atc)
                    for t in range(2):
                        po = t * Dh
                        rz = spool.tile([1, IB], F32, tag="rz", name="rz")
                        nc.vector.reciprocal(rz[:], atcs[t][Dh : Dh + 1, :])
                        bc = spool.tile([Dh, IB], F32, tag="bc", name="bc")
                        nc.gpsimd.partition_broadcast(bc[:], rz[:])
                        nc.vector.tensor_tensor(
                            aTs[ib][po : po + Dh, fc, :],
                            atcs[t][0:Dh, :],
                            bc[:],
                            mybir.AluOpType.mult,
                        )

                if ib + 1 < NIB:
                    q_proj(ib + 1)
                if ib > 0:
                    out_proj(ib - 1)
            out_proj(NIB - 1)

    nc.compile()
    return nc


def kernel(key, query, value, Wq, bq, Wk, bk, Wv, bv, Wo, bo):
    global last_exec_time_ns, last_results, _built
    key = np.asarray(key, np.float32)
    query = np.asarray(query, np.float32)
    value = np.asarray(value, np.float32)
    Wq = np.asarray(Wq, np.float32)
    Wk = np.asarray(Wk, np.float32)
    Wv = np.asarray(Wv, np.float32)
    Wo = np.asarray(Wo, np.float32)
    bq = np.asarray(bq, np.float32)
    bv = np.asarray(bv, np.float32)
    bo = np.asarray(bo, np.float32)

    s = 1.0 / math.sqrt(Dh)
    xqT = [np.ascontiguousarray(query[b].T) for b in range(B)]
    xkT = [np.ascontiguousarray(key[b].T) for b in range(B)]
    xvT = [np.ascontiguousarray(value[b].T) for b in range(B)]
    WqTs = np.ascontiguousarray(Wq.T) * s
    WkT = np.ascontiguousarray(Wk.T)
    WvT = np.ascontiguousarray(Wv.T)
    WoT = np.ascontiguousarray(Wo.T)

    in_maps = []
    for c in range(NCORES):
        b, g = c // GPB, c % GPB
        fsl = slice(g * F, (g + 1) * F)
        in_maps.append(
            {
                "xq": xqT[b],
                "xk": xkT[b],
                "xv": xvT[b],
                "wqt": np.ascontiguousarray(WqTs[:, fsl]),
                "wkt": np.ascontiguousarray(WkT[:, fsl]),
                "wvt": np.ascontiguousarray(WvT[:, fsl]),
                "wot": np.ascontiguousarray(WoT[fsl, :]).astype(ml_dtypes.bfloat16),
                "bqv": np.ascontiguousarray((bq[fsl] * s).reshape(1, F)),
            }
        )

    if _built is None:
        _built = _build()
    trace = bool(int(os.environ.get("KERNEL_TRACE", "0")))
    res = run_bass_kernel_spmd(
        _built, in_maps, core_ids=list(range(NCORES)), trace=trace
    )
    last_exec_time_ns = res.exec_time_ns
    last_results = res
    parts = [r["out"] for r in res.results]

    outs = np.stack(
        [sum(parts[b * GPB + g] for g in range(GPB)) for b in range(B)]
    ).astype(np.float32)
    outs += (bv @ Wo.T + bo)[None, None, :]
    return outs
